# revision 1
# baseline (speedup 1.0000x reference)
"""Trainium2 Bass kernel for GraphPoolingLayer: softmax(x @ W + b, axis=1)
followed by segment_sum over sorted segment ids.

Sharding: segments are split into 8 contiguous ranges (6250 per core); each
core takes every atom row belonging to its segment range, so per-segment
partial sums never cross cores and the full output is a concatenation.

Device-side algorithm, per 128-row chunk:
  lin[r,o]  = matmul(lhsT=xT_chunk, rhs=W)          (PE -> PSUM, bias pre-folded
                                                     into x on host via b @ W^-1)
  y[r,o]    = exp(lin)                               (ACT, wide over 8 chunks)
  s[r]      = sum_o y[r,o] ; recip = 1/s             (DVE, wide)
  oh[r,j]   = (iota[j] == rel[r]) * recip[r]         (DVE tensor_scalar, fused)
  out[seg,o] += matmul(lhsT=oh, rhs=y)               (PE, PSUM-accumulated over
                                                     all chunks of a 128-seg group)
Rows are padded on host into a uniform per-group chunk grid (identical across
cores so one SPMD program serves all 8); pad rows carry rel=-1 so their onehot
column is all-zero and they contribute nothing.
"""

import numpy as np

import concourse.bass as bass
import concourse.bacc as bacc
import concourse.mybir as mybir
import concourse.tile as tile
from concourse.bass_utils import run_bass_kernel_spmd

N_ATOMS = 1_000_000
N_MOLS = 50_000
D = 128
NDEV = 8
SPD = N_MOLS // NDEV          # segments per device
G = -(-SPD // 128)            # 128-segment groups per device
P = 128
B = 8                         # chunks per macro tile (DMA/exp/reduce batch)

F32 = mybir.dt.float32
BF16 = mybir.dt.bfloat16

# Set by build_nc; toggles reduced-precision path for the on-chip tensors that
# feed the PE (xT / y / onehot) to halve DMA + get DVE fast modes.
USE_BF16_X = True
USE_BF16_Y = True

_compiled = {}


def _layout(segment_ids):
    """Compute the shared chunk grid from the (sorted) segment ids."""
    seg = np.asarray(segment_ids)
    dev_rows = np.searchsorted(seg, np.arange(0, N_MOLS + 1, SPD))
    cnt = np.empty((NDEV, G), dtype=np.int64)
    for d in range(NDEV):
        edges = np.minimum(d * SPD + np.arange(0, G * 128 + 128, 128), (d + 1) * SPD)
        cnt[d] = np.diff(np.searchsorted(seg, edges))
    cpg = -(-cnt.max(axis=0) // 128)          # chunks per group, shared by all devices
    cpg = np.maximum(cpg, 1)
    T = int(cpg.sum())
    chunk_grp = np.repeat(np.arange(G), cpg)  # group index of each chunk
    grp_chunk0 = np.concatenate([[0], np.cumsum(cpg)])[:-1]
    return dev_rows, cnt, cpg, T, chunk_grp, grp_chunk0


def build_nc(T, chunk_grp):
    nc = bacc.Bacc("TRN2", target_bir_lowering=False, debug=False)
    x_dt = BF16 if USE_BF16_X else F32
    y_dt = BF16 if USE_BF16_Y else F32

    xT = nc.dram_tensor("xT", [P, T * P], x_dt, kind="ExternalInput")
    rel = nc.dram_tensor("rel", [P, T], F32, kind="ExternalInput")
    w_in = nc.dram_tensor("W", [D, D], x_dt, kind="ExternalInput")
    iota_in = nc.dram_tensor("iota", [P, P], BF16 if USE_BF16_Y else F32, kind="ExternalInput")
    out = nc.dram_tensor("out", [G * P, D], F32, kind="ExternalOutput")

    n_macro = -(-T // B)

    with tile.TileContext(nc) as tc:
        with (
            tc.tile_pool(name="const", bufs=1) as cpool,
            tc.tile_pool(name="xsb", bufs=3) as xpool,
            tc.tile_pool(name="ysb", bufs=3) as ypool,
            tc.tile_pool(name="ohsb", bufs=12) as ohpool,
            tc.tile_pool(name="stat", bufs=4) as spool,
            tc.tile_pool(name="plin", bufs=2, space="PSUM") as plin_pool,
            tc.tile_pool(name="pout", bufs=3, space="PSUM") as pout_pool,
        ):
            w_sb = cpool.tile([D, D], x_dt)
            nc.sync.dma_start(w_sb[:], w_in[:])
            iota_sb = cpool.tile([P, P], BF16 if USE_BF16_Y else F32)
            nc.sync.dma_start(iota_sb[:], iota_in[:])
            rel_sb = cpool.tile([P, T], F32)
            nc.sync.dma_start(rel_sb[:], rel[:])

            psum_out = None
            for m in range(n_macro):
                c0 = m * B
                nb = min(B, T - c0)
                x_sb = xpool.tile([P, B * P], x_dt, tag="x")
                nc.sync.dma_start(
                    x_sb[:, : nb * P], xT[:, c0 * P : (c0 + nb) * P]
                )
                lin = plin_pool.tile([P, B * P], F32, space="PSUM", tag="lin")
                for k in range(nb):
                    nc.tensor.matmul(
                        lin[:, k * P : (k + 1) * P],
                        lhsT=x_sb[:, k * P : (k + 1) * P],
                        rhs=w_sb[:],
                        start=True,
                        stop=True,
                    )
                y_sb = ypool.tile([P, B * P], y_dt, tag="y")
                nc.scalar.activation(
                    y_sb[:, : nb * P], lin[:, : nb * P],
                    mybir.ActivationFunctionType.Exp,
                )
                s_sb = spool.tile([P, B], F32, tag="s")
                nc.vector.reduce_sum(
                    s_sb[:, :nb],
                    y_sb[:, : nb * P].rearrange("p (c f) -> p c f", f=P),
                    axis=mybir.AxisListType.X,
                )
                r_sb = spool.tile([P, B], F32, tag="r")
                nc.vector.reciprocal(r_sb[:, :nb], s_sb[:, :nb])

                for k in range(nb):
                    c = c0 + k
                    g = chunk_grp[c]
                    oh = ohpool.tile([P, P], y_dt, tag="oh")
                    nc.vector.tensor_scalar(
                        oh[:],
                        iota_sb[:],
                        rel_sb[:, c : c + 1],
                        r_sb[:, k : k + 1],
                        op0=mybir.AluOpType.is_equal,
                        op1=mybir.AluOpType.mult,
                    )
                    first = c == 0 or chunk_grp[c - 1] != g
                    last = c == T - 1 or chunk_grp[c + 1] != g
                    if first:
                        psum_out = pout_pool.tile([P, D], F32, space="PSUM", tag="po")
                    nc.tensor.matmul(
                        psum_out[:],
                        lhsT=oh[:],
                        rhs=y_sb[:, k * P : (k + 1) * P],
                        start=first,
                        stop=last,
                    )
                    if last:
                        o_sb = ohpool.tile([P, D], F32, tag="osb")
                        nc.vector.tensor_copy(o_sb[:], psum_out[:])
                        nc.sync.dma_start(out[g * P : (g + 1) * P, :], o_sb[:])
    nc.compile()
    return nc


def kernel(atom_features, segment_ids, W, b):
    x = np.asarray(atom_features, dtype=np.float32)
    seg = np.asarray(segment_ids)
    W = np.asarray(W, dtype=np.float32)
    b = np.asarray(b, dtype=np.float32)

    dev_rows, cnt, cpg, T, chunk_grp, grp_chunk0 = _layout(seg)

    key = (T, tuple(cpg.tolist()))
    if key not in _compiled:
        _compiled[key] = build_nc(T, chunk_grp)
    nc = _compiled[key]

    # fold bias into x: (x + c) @ W = x @ W + b  with  c = b @ W^-1
    c_row = np.linalg.solve(W.astype(np.float64).T, b.astype(np.float64))
    c_row = c_row.astype(np.float32)

    x_np_dt = np.float32
    if USE_BF16_X:
        import ml_dtypes
        x_np_dt = ml_dtypes.bfloat16

    iota_np_dt = np.float32
    if USE_BF16_Y:
        import ml_dtypes
        iota_np_dt = ml_dtypes.bfloat16
    iota = np.broadcast_to(np.arange(P).astype(iota_np_dt), (P, P)).copy()

    in_maps = []
    for d in range(NDEV):
        r0, r1 = dev_rows[d], dev_rows[d + 1]
        x_dev = x[r0:r1] + c_row
        seg_dev = seg[r0:r1].astype(np.int64) - d * SPD

        # position of each real row in the padded chunk grid
        n_dev = r1 - r0
        grp_of_row = seg_dev >> 7                       # 128 segs per group
        cnt_d = cnt[d]
        start_of_grp = grp_chunk0 * 128                 # padded start row of group
        within = np.arange(n_dev) - np.concatenate([[0], np.cumsum(cnt_d)])[:-1][grp_of_row]
        pos = start_of_grp[grp_of_row] + within

        xpad = np.zeros((T * 128, D), dtype=np.float32)
        xpad[pos] = x_dev
        relpad = np.full(T * 128, -1.0, dtype=np.float32)
        relpad[pos] = (seg_dev & 127).astype(np.float32)

        in_maps.append({
            "xT": np.ascontiguousarray(xpad.T).astype(x_np_dt),
            "rel": np.ascontiguousarray(relpad.reshape(T, 128).T),
            "W": W.astype(x_np_dt),
            "iota": iota,
        })

    global LAST_NC, LAST_IN_MAPS
    LAST_NC, LAST_IN_MAPS = nc, in_maps
    res = run_bass_kernel_spmd(nc, in_maps, list(range(NDEV)))
    global LAST_RESULTS
    LAST_RESULTS = res
    out = np.concatenate([res.results[d]["out"][:SPD] for d in range(NDEV)], axis=0)
    return np.ascontiguousarray(out, dtype=np.float32)


LAST_RESULTS = None
LAST_NC = None
LAST_IN_MAPS = None



# revision 2
# speedup vs baseline: 1.1298x; 1.1298x over previous
"""Trainium2 Bass kernel V2 for GraphPoolingLayer: softmax(x @ W + b, axis=1)
followed by segment_sum over sorted segment ids.

Sharding: segments split into 8 contiguous ranges (6250/core); each core takes
every atom row in its segment range; output is a concatenation.

V2 engine allocation (per 12-chunk macro, 4-macro supers for the row-sum tree):
- DMA: x in fp8_e4m3 (W bf16 -> ~9e-3 norm err), rel f32, out writes
- PE: GEMM1 per chunk (x stationary fp8, W moving bf16) + GEMM2 per chunk
  (onehot stationary, y moving, PSUM-accumulated per 128-segment group)
- ACT: exp over [128, 12*128] macros + PSUM->SBUF output copies
- DVE: onehot build (iota==rel)*recip, z2/z3/tail of the row-sum tree
  (batched per super), reciprocal
- Pool(GpSimd): add1 halving of y (SBUF only; fraction tunable)
"""

import numpy as np

import concourse.bass as bass
import concourse.bacc as bacc
import concourse.mybir as mybir
import concourse.tile as tile
from concourse.bass_utils import run_bass_kernel_spmd

N_ATOMS = 1_000_000
N_MOLS = 50_000
D = 128
NDEV = 8
SPD = N_MOLS // NDEV          # segments per device
G = -(-SPD // 128)            # 128-segment groups per device
P = 128
B = 12                        # chunks per macro (DMA/exp batch; PSUM 3 banks x 2)
SB = 2                        # macros per super (row-sum tree batch)

F32 = mybir.dt.float32
BF16 = mybir.dt.bfloat16
FP8 = mybir.dt.float8e4

ADD1_POOL_FRAC = 1.0          # fraction of macros whose add1 runs on Pool
OUTCOPY = "act"               # engine for PSUM->SBUF output copy
OHBUFS = 24
POUTB = 2

_compiled = {}


def _layout(segment_ids):
    seg = np.asarray(segment_ids)
    dev_rows = np.searchsorted(seg, np.arange(0, N_MOLS + 1, SPD))
    cnt = np.empty((NDEV, G), dtype=np.int64)
    for d in range(NDEV):
        edges = np.minimum(d * SPD + np.arange(0, G * 128 + 128, 128), (d + 1) * SPD)
        cnt[d] = np.diff(np.searchsorted(seg, edges))
    cpg = -(-cnt.max(axis=0) // 128)
    cpg = np.maximum(cpg, 1)
    T = int(cpg.sum())
    chunk_grp = np.repeat(np.arange(G), cpg)
    grp_chunk0 = np.concatenate([[0], np.cumsum(cpg)])[:-1]
    return dev_rows, cnt, cpg, T, chunk_grp, grp_chunk0


def build_nc(T, chunk_grp):
    nc = bacc.Bacc("TRN2", target_bir_lowering=False, debug=False)

    xT = nc.dram_tensor("xT", [P, T * P], FP8, kind="ExternalInput")
    rel = nc.dram_tensor("rel", [P, T], F32, kind="ExternalInput")
    w_in = nc.dram_tensor("W", [D, D], BF16, kind="ExternalInput")
    iota_in = nc.dram_tensor("iota", [P, P], BF16, kind="ExternalInput")
    out = nc.dram_tensor("out", [G * P, D], F32, kind="ExternalOutput")

    n_macro = -(-T // B)
    n_super = -(-n_macro // SB)

    with tile.TileContext(nc) as tc:
        with (
            tc.tile_pool(name="const", bufs=1) as cpool,
            tc.tile_pool(name="xsb", bufs=3) as xpool,
            tc.tile_pool(name="ysb", bufs=3 * SB + 2) as ypool,
            tc.tile_pool(name="z1sb", bufs=2) as z1pool,
            tc.tile_pool(name="z2sb", bufs=2) as z2pool,
            tc.tile_pool(name="ohsb", bufs=OHBUFS) as ohpool,
            tc.tile_pool(name="stat", bufs=4) as spool,
            tc.tile_pool(name="osb", bufs=3) as opool,
            tc.tile_pool(name="plin", bufs=2, space="PSUM") as plin_pool,
            tc.tile_pool(name="pout", bufs=POUTB, space="PSUM") as pout_pool,
        ):
            w_sb = cpool.tile([D, D], BF16)
            nc.sync.dma_start(w_sb[:], w_in[:])
            iota_sb = cpool.tile([P, P], BF16)
            nc.sync.dma_start(iota_sb[:], iota_in[:])
            rel_sb = cpool.tile([P, T], F32)
            nc.sync.dma_start(rel_sb[:], rel[:])

            psum_state = [None]

            def gemm2_block(y_tiles, c_sp, r_sb):
                """Onehot + segment-sum matmuls + group output copies for one super."""
                for y_sb, c0, nb in y_tiles:
                    for k in range(nb):
                        c = c0 + k
                        g = chunk_grp[c]
                        oh = ohpool.tile([P, P], BF16, tag="oh")
                        nc.vector.tensor_scalar(
                            oh[:],
                            iota_sb[:],
                            rel_sb[:, c : c + 1],
                            r_sb[:, c - c_sp : c - c_sp + 1],
                            op0=mybir.AluOpType.is_equal,
                            op1=mybir.AluOpType.mult,
                        )
                        first = c == 0 or chunk_grp[c - 1] != g
                        last = c == T - 1 or chunk_grp[c + 1] != g
                        if first:
                            psum_state[0] = pout_pool.tile([P, D], F32, space="PSUM", tag="po", name="po")
                        nc.tensor.matmul(
                            psum_state[0][:],
                            lhsT=oh[:],
                            rhs=y_sb[:, k * P : (k + 1) * P],
                            start=first,
                            stop=last,
                        )
                        if last:
                            o_sb = opool.tile([P, D], F32, tag="osb")
                            if OUTCOPY == "act":
                                nc.scalar.activation(o_sb[:], psum_state[0][:], mybir.ActivationFunctionType.Copy)
                            else:
                                nc.vector.tensor_copy(o_sb[:], psum_state[0][:])
                            nc.sync.dma_start(out[g * P : (g + 1) * P, :], o_sb[:])

            pending = None
            for sp in range(n_super):
                m0 = sp * SB
                n_m = min(SB, n_macro - m0)
                c_sp = m0 * B                      # first chunk of super
                nb_sp = min(SB * B, T - c_sp)      # chunks in super

                # z1: add1 output for the whole super [P, nb_sp*64] bf16
                z1 = z1pool.tile([P, SB * B * 64], BF16, tag="z1")
                y_tiles = []
                for mi in range(n_m):
                    m = m0 + mi
                    c0 = m * B
                    nb = min(B, T - c0)
                    x_sb = xpool.tile([P, B * P], FP8, tag="x")
                    nc.sync.dma_start(x_sb[:, : nb * P], xT[:, c0 * P : (c0 + nb) * P])
                    lin = plin_pool.tile([P, B * P], F32, space="PSUM", tag="lin")
                    for k in range(nb):
                        nc.tensor.matmul(
                            lin[:, k * P : (k + 1) * P],
                            lhsT=x_sb[:, k * P : (k + 1) * P],
                            rhs=w_sb[:],
                            start=True,
                            stop=True,
                        )
                    y_sb = ypool.tile([P, B * P], BF16, tag="y")
                    nc.scalar.activation(
                        y_sb[:, : nb * P], lin[:, : nb * P],
                        mybir.ActivationFunctionType.Exp,
                    )
                    y_tiles.append((y_sb, c0, nb))
                    # add1: y[c, 0:64] + y[c, 64:128] -> z1[mi*B + c, 0:64]
                    yv = y_sb[:, : nb * P].rearrange("p (c f) -> p c f", f=P)
                    z1v = z1[:, (mi * B) * 64 : (mi * B + nb) * 64].rearrange(
                        "p (c f) -> p c f", f=64
                    )
                    eng = nc.gpsimd if (m % 1000) < ADD1_POOL_FRAC * 1000 else nc.vector
                    eng.tensor_tensor(z1v, yv[:, :, 0:64], yv[:, :, 64:128], op=mybir.AluOpType.add)

                # super-batched tree tail on DVE
                z1v_all = z1[:, : nb_sp * 64].rearrange("p (c f) -> p c f", f=64)
                z2 = z2pool.tile([P, SB * B * 32], BF16, tag="z2")
                z2v = z2[:, : nb_sp * 32].rearrange("p (c f) -> p c f", f=32)
                nc.vector.tensor_tensor(z2v, z1v_all[:, :, 0:32], z1v_all[:, :, 32:64], op=mybir.AluOpType.add)
                z3 = z2pool.tile([P, SB * B * 16], BF16, tag="z3")
                z3v = z3[:, : nb_sp * 16].rearrange("p (c f) -> p c f", f=16)
                nc.vector.tensor_tensor(z3v, z2v[:, :, 0:16], z2v[:, :, 16:32], op=mybir.AluOpType.add)
                s_sb = spool.tile([P, SB * B], F32, tag="s")
                nc.vector.reduce_sum(s_sb[:, :nb_sp], z3v, axis=mybir.AxisListType.X)
                r_sb = spool.tile([P, SB * B], F32, tag="r")
                nc.vector.reciprocal(r_sb[:, :nb_sp], s_sb[:, :nb_sp])

                # software pipeline: emit the previous super's GEMM2 block now,
                # so its PE/DVE work overlaps this super's loads/exps
                if pending is not None:
                    gemm2_block(*pending)
                pending = (y_tiles, c_sp, r_sb)
            if pending is not None:
                gemm2_block(*pending)
    nc.compile()
    return nc


def kernel(atom_features, segment_ids, W, b):
    import ml_dtypes

    x = np.asarray(atom_features, dtype=np.float32)
    seg = np.asarray(segment_ids)
    W = np.asarray(W, dtype=np.float32)
    b = np.asarray(b, dtype=np.float32)

    dev_rows, cnt, cpg, T, chunk_grp, grp_chunk0 = _layout(seg)

    key = (T, tuple(cpg.tolist()))
    if key not in _compiled:
        _compiled[key] = build_nc(T, chunk_grp)
    nc = _compiled[key]

    # fold bias into x: (x + c) @ W = x @ W + b  with  c = b @ W^-1
    c_row = np.linalg.solve(W.astype(np.float64).T, b.astype(np.float64))
    c_row = c_row.astype(np.float32)

    iota = np.broadcast_to(np.arange(P).astype(ml_dtypes.bfloat16), (P, P)).copy()

    in_maps = []
    for d in range(NDEV):
        r0, r1 = dev_rows[d], dev_rows[d + 1]
        x_dev = x[r0:r1] + c_row
        seg_dev = seg[r0:r1].astype(np.int64) - d * SPD

        n_dev = r1 - r0
        grp_of_row = seg_dev >> 7
        cnt_d = cnt[d]
        start_of_grp = grp_chunk0 * 128
        within = np.arange(n_dev) - np.concatenate([[0], np.cumsum(cnt_d)])[:-1][grp_of_row]
        pos = start_of_grp[grp_of_row] + within

        xpad = np.zeros((T * 128, D), dtype=np.float32)
        xpad[pos] = x_dev
        relpad = np.full(T * 128, -1.0, dtype=np.float32)
        relpad[pos] = (seg_dev & 127).astype(np.float32)

        in_maps.append({
            "xT": np.ascontiguousarray(xpad.T).astype(ml_dtypes.float8_e4m3),
            "rel": np.ascontiguousarray(relpad.reshape(T, 128).T),
            "W": W.astype(ml_dtypes.bfloat16),
            "iota": iota,
        })

    global LAST_NC, LAST_IN_MAPS
    LAST_NC, LAST_IN_MAPS = nc, in_maps
    res = run_bass_kernel_spmd(nc, in_maps, list(range(NDEV)))
    global LAST_RESULTS
    LAST_RESULTS = res
    out = np.concatenate([res.results[d]["out"][:SPD] for d in range(NDEV)], axis=0)
    return np.ascontiguousarray(out, dtype=np.float32)


LAST_RESULTS = None
LAST_NC = None
LAST_IN_MAPS = None
